# revision 28
# baseline (speedup 1.0000x reference)
# Discrete-Hawkes kernel, v4: windowed-decay direct query evaluation.
#
# lam(t,s) = relu( mu[s] + beta * H[t,s] ),
#   H[t,s] = sum_{tp<t} a^(t-tp) * c[tp,s],  c = obs @ alpha,  a = exp(-beta)
#
# Key identity: swap the contraction order. Only B = 8192 grid points are
# read, so never form c/H on the [T,S] grid:
#
#   out[i] = relu( mu[s_i] + sum_j sum_sp G[i,j] * obs[j,sp] * alpha[sp,s_i] )
#   G[i,j] = beta * a^(t_i - j)  for 0 < t_i - j, truncated to a >=64-step
#            window (a^64 = exp(-64*beta) <= 1.7e-3 relative, beta >= 0.1).
#
# Per core (1024 queries, sorted by t):
#  * Q = G^T-matmuls over a 10-tile obs slab: 9 query tiles x 3 j-tiles x
#    2 PSUM halves (bf16 G^T stationary, fp8 obs moving -- mixed dtypes).
#  * out[i] = relu(dot(Q[i,:], alphaT[s_i,:]) + mu[s_i]): per tile one
#    elementwise product (DVE / Pool alternating) + a free-axis accumulate
#    (ScalarE activation accumulator / DVE tensor_reduce alternating).
#  * All gathers (G rows by t_i shift, alphaT rows by s_i, mu by s_i) and
#    the final inverse permutation are host-side index prep; the G decay
#    table folds in beta so no runtime scalars reach the device.
#
# DMA: everything fp8 except G^T (bf16, 0.88MB); ~3.4MB/core over three
# issue queues (sync/scalar HWDGE + gpsimd SWDGE), split into small
# tensors ordered so each query tile's gates land just in time. A stream
# of tiny dummy matmuls keeps the PE p-state ramped while inputs land.

import numpy as np
import ml_dtypes

T, S, B = 8192, 1024, 8192
NCORES = 8
P = 128
WIN = 64            # guaranteed decay lookback per query
JW = 384            # G row width = 3 j-tiles
QT = 9              # query tiles per core (1152 slots for 1024 queries)
ANCH = (0, 1, 2, 3, 4, 5, 6, 7, 7)   # j-tile anchor per query tile
NJ = 10             # obs j-tiles per core (1280 rows)
NQ = B // NCORES    # queries per core
N_DUMMY = 48        # PE p-state keep-alive matmuls (tiny)

# tensor splits for fine-grained DMA gating
GT_SPLITS = ((0, 3), (3, 5), (5, 9))        # q-tile ranges (sync,sync,scalar)
OBS_SPLITS = ((0, 3), (3, 5), (5, 10))      # j-tile ranges (sync,sync,scalar)
AG_SPLITS = ((0, 3), (3, 6), (6, 9))        # q-tile ranges (gpsimd)
DVE_RED = (4, 8)    # q-tiles whose accumulate runs as a DVE reduce

_NC_CACHE = {}
LAST_RESULT = None


def _build():
    if "nc" in _NC_CACHE:
        return _NC_CACHE["nc"]

    import concourse.mybir as mybir
    import concourse.tile as tile
    from concourse import bacc

    dt = mybir.dt
    nc = bacc.Bacc("TRN2", target_bir_lowering=False, debug=False,
                   num_devices=NCORES)

    gt_d = [nc.dram_tensor(f"gt{i}", [P, b - a, 3, P], dt.bfloat16,
                           kind="ExternalInput")
            for i, (a, b) in enumerate(GT_SPLITS)]
    obs_d = [nc.dram_tensor(f"obs{i}", [P, b - a, S], dt.float8e4,
                            kind="ExternalInput")
             for i, (a, b) in enumerate(OBS_SPLITS)]
    ag_d = [nc.dram_tensor(f"ag{i}", [P, b - a, S], dt.float8e4,
                           kind="ExternalInput")
            for i, (a, b) in enumerate(AG_SPLITS)]
    fl_d = nc.dram_tensor("fl", [P, QT], dt.float32, kind="ExternalInput")
    lam_d = nc.dram_tensor("lam", [P, QT], dt.float32, kind="ExternalOutput")

    with tile.TileContext(nc) as tc:
        with (
            tc.tile_pool(name="inp", bufs=1) as inp,
            tc.tile_pool(name="psq", bufs=3, space="PSUM") as psq_pool,
            tc.tile_pool(name="psd", bufs=1, space="PSUM") as psd_pool,
            tc.tile_pool(name="scr", bufs=4) as scr_pool,
            tc.tile_pool(name="acc", bufs=QT) as acc_pool,
        ):
            dmy = inp.tile([P, P], dt.bfloat16, tag="dmy")
            nc.vector.memset(dmy[:], 0)
            psd = psd_pool.tile([P, P], dt.float32, tag="psd")

            fl_sb = inp.tile([P, QT], dt.float32, tag="fl")
            gt_sb = [inp.tile([P, b - a, 3, P], dt.bfloat16, tag=f"gt{i}",
                              name=f"gt{i}")
                     for i, (a, b) in enumerate(GT_SPLITS)]
            obs_sb = [inp.tile([P, b - a, S], dt.float8e4, tag=f"obs{i}",
                               name=f"obs{i}")
                      for i, (a, b) in enumerate(OBS_SPLITS)]
            ag_sb = [inp.tile([P, b - a, S], dt.float8e4, tag=f"ag{i}",
                              name=f"ag{i}")
                     for i, (a, b) in enumerate(AG_SPLITS)]
            lam_sb = inp.tile([P, QT], dt.float32, tag="lam")
            acc_sb = inp.tile([P, QT], dt.float32, tag="acc")
            scr2 = inp.tile([P, S], dt.bfloat16, tag="scr2")

            # Three DMA queues in parallel; critical gates first on each.
            # sync: gt0 -> obs0 -> gt1 -> obs1; scalar: gt2 -> obs2;
            # gpsimd (SWDGE): fl -> ag chunks.
            nc.sync.dma_start(gt_sb[0][:], gt_d[0][:])
            nc.scalar.dma_start(gt_sb[2][:], gt_d[2][:])
            nc.gpsimd.dma_start(fl_sb[:], fl_d[:])
            nc.sync.dma_start(obs_sb[0][:], obs_d[0][:])
            nc.scalar.dma_start(obs_sb[2][:], obs_d[2][:])
            nc.gpsimd.dma_start(ag_sb[0][:], ag_d[0][:])
            nc.sync.dma_start(gt_sb[1][:], gt_d[1][:])
            nc.sync.dma_start(obs_sb[1][:], obs_d[1][:])
            nc.gpsimd.dma_start(ag_sb[1][:], ag_d[1][:])
            nc.gpsimd.dma_start(ag_sb[2][:], ag_d[2][:])

            # PE clock keep-alive while inputs land: many tiny matmuls, so
            # the first real matmul slots in with little queue delay.
            for _ in range(N_DUMMY):
                nc.tensor.matmul(psd[:, :], dmy[:, :], dmy[:, :],
                                 start=True, stop=True)

            def obs_ap(jt):
                for i, (a, b) in enumerate(OBS_SPLITS):
                    if a <= jt < b:
                        return obs_sb[i], jt - a
                raise AssertionError

            def ag_ap(g):
                for i, (a, b) in enumerate(AG_SPLITS):
                    if a <= g < b:
                        return ag_sb[i], g - a
                raise AssertionError

            def gt_ap(g):
                for i, (a, b) in enumerate(GT_SPLITS):
                    if a <= g < b:
                        return gt_sb[i], g - a
                raise AssertionError

            def qtile(g):
                gt, gl = gt_ap(g)
                psq = psq_pool.tile([P, S], dt.float32, tag="psq",
                                    name=f"psq{g}")
                # PSUM matmul groups are per-bank: two 512-wide halves.
                for h in range(2):
                    for c in range(3):
                        ob, jl = obs_ap(ANCH[g] + c)
                        nc.tensor.matmul(
                            psq[:, h * 512:(h + 1) * 512],
                            gt[:, gl, c, :],
                            ob[:, jl, h * 512:(h + 1) * 512],
                            start=(c == 0), stop=(c == 2))
                scr = scr_pool.tile([P, S], dt.bfloat16, tag="scr",
                                    name=f"scr{g}")
                agt, al = ag_ap(g)
                # dot(Q[i,:], alphaT[s_i,:]): product on DVE (only engine
                # that can read PSUM fast); accumulate on ScalarE's
                # activation accumulator, two tiles offloaded to DVE.
                nc.vector.tensor_tensor(out=scr[:], in0=psq[:],
                                        in1=agt[:, al, :],
                                        op=mybir.AluOpType.mult)
                if g in DVE_RED:
                    nc.vector.tensor_reduce(out=acc_sb[:, g:g + 1],
                                            in_=scr[:],
                                            axis=mybir.AxisListType.X,
                                            op=mybir.AluOpType.add)
                else:
                    nc.scalar.activation(scr2[:], scr[:],
                                         mybir.ActivationFunctionType.Copy,
                                         accum_out=acc_sb[:, g:g + 1])

            for g in range(QT):
                qtile(g)

            # lam = relu(acc + mu[s_i]) for all tiles at once (beta is
            # folded into G); two small DVE ops instead of 9 activations.
            nc.vector.tensor_tensor(out=lam_sb[:], in0=acc_sb[:],
                                    in1=fl_sb[:], op=mybir.AluOpType.add)
            nc.vector.tensor_scalar_max(lam_sb[:], lam_sb[:], 0.0)
            nc.sync.dma_start(lam_d[:], lam_sb[:])

    nc.compile()
    _NC_CACHE["nc"] = nc
    return nc


def _assign(tc):
    """Greedy assignment of one core's sorted t values to the QT tiles.

    Tile g may hold query t iff its 384-wide obs window [A0+128*ANCH[g],
    ... +JW) covers [max(0, t-WIN), t]. Returns (A0, per-tile index lists).
    """
    tmin, tmax = int(tc[0]), int(tc[-1])
    span = tmax - tmin
    top = 128 * ANCH[-1] + JW - 1
    cands = [tmin - WIN - max(0, (top - WIN - span)) // 2]
    cands += list(range(tmax - top, tmin - WIN + 1, 32))
    for A0 in cands:
        if tmax > A0 + top:
            continue
        assign = [[] for _ in range(QT)]
        g = 0
        ok = True
        for qi in range(len(tc)):
            tv = int(tc[qi])
            while g < QT and not (
                    max(0, tv - WIN) >= A0 + 128 * ANCH[g]
                    and tv <= A0 + 128 * ANCH[g] + JW - 1
                    and len(assign[g]) < P):
                g += 1
            if g == QT:
                ok = False
                break
            assign[g].append(qi)
        if ok:
            return A0, assign
    raise RuntimeError("no feasible window placement for this t distribution")


def _prep_inputs(t, s, obs, alpha, beta, mu):
    bf16 = ml_dtypes.bfloat16
    fp8 = ml_dtypes.float8_e4m3fn
    t_i = np.asarray(t).astype(np.int64)
    s_i = np.asarray(s).astype(np.int64)
    beta32 = np.float32(np.asarray(beta).reshape(-1)[0])
    a64 = np.exp(-np.float64(beta32))

    # Decay table, one row per shift u: gtab[u, j] = beta * a^(384-u-j) for
    # 1 <= 384-u-j < 384, else 0. A query t in tile g (core offset A0) uses
    # row u = 384 - (t - A0 - 128*ANCH[g]); row 384 is all zero (pad slots).
    uu = np.arange(385, dtype=np.int64)[:, None]
    jj = np.arange(JW, dtype=np.int64)[None, :]
    x = 384 - uu - jj
    tab = np.where((x >= 1) & (x < 384), a64 ** np.clip(x, 0, 400), 0.0)
    gtab = (beta32 * tab).astype(np.float32).astype(bf16)

    obs_f8 = np.asarray(obs).astype(fp8)
    alphat = np.ascontiguousarray(
        np.asarray(alpha, dtype=np.float32).T).astype(fp8)
    mu32 = np.asarray(mu, dtype=np.float32)

    order = np.argsort(t_i, kind="stable")
    in_maps = []
    slot_q = np.full((NCORES, P, QT), -1, dtype=np.int64)
    for k in range(NCORES):
        idx = order[k * NQ:(k + 1) * NQ]
        tc, sc = t_i[idx], s_i[idx]
        A0, assign = _assign(tc)

        block = np.zeros((NJ * P, S), dtype=fp8)
        lo, hi = max(0, A0), min(T, A0 + NJ * P)
        block[lo - A0:hi - A0] = obs_f8[lo:hi]
        slab = block.reshape(NJ, P, S).transpose(1, 0, 2)

        uoff = np.full((P, QT), 384, dtype=np.int64)  # 384 = all-zero row
        soff = np.zeros((P, QT), dtype=np.int64)
        fl = np.zeros((P, QT), dtype=np.float32)
        for g, lst in enumerate(assign):
            for p, qi in enumerate(lst):
                tv, sv = int(tc[qi]), int(sc[qi])
                uoff[p, g] = 384 - (tv - A0 - 128 * ANCH[g])
                soff[p, g] = sv
                fl[p, g] = mu32[sv]
                slot_q[k, p, g] = idx[qi]
        g_rows = gtab[uoff]                       # [P(q), QT, JW]
        ag_rows = alphat[soff]                    # [P, QT, S]
        ag_rows[uoff == 384] = 0                  # pad slots contribute 0
        # pre-transposed G chunks: gt[j, g, c, q] = g_rows[q, g, c*128+j]
        gt = g_rows.reshape(P, QT, 3, P).transpose(3, 1, 2, 0)

        im = {"fl": fl}
        for i, (a, b) in enumerate(GT_SPLITS):
            im[f"gt{i}"] = np.ascontiguousarray(gt[:, a:b])
        for i, (a, b) in enumerate(OBS_SPLITS):
            im[f"obs{i}"] = np.ascontiguousarray(slab[:, a:b])
        for i, (a, b) in enumerate(AG_SPLITS):
            im[f"ag{i}"] = np.ascontiguousarray(ag_rows[:, a:b])
        in_maps.append(im)
    return in_maps, slot_q


def kernel(t, s, obs, alpha, beta, mu):
    global LAST_RESULT
    from concourse import bass_utils

    nc = _build()
    in_maps, slot_q = _prep_inputs(t, s, obs, alpha, beta, mu)
    res = bass_utils.run_bass_kernel_spmd(nc, in_maps,
                                          core_ids=list(range(NCORES)))
    LAST_RESULT = res

    lam = np.stack([np.asarray(r["lam"], dtype=np.float32)
                    for r in res.results])        # [NCORES, P, QT]
    out = np.zeros(B, dtype=np.float32)
    valid = slot_q >= 0
    out[slot_q[valid]] = lam[valid]
    return np.ascontiguousarray(out)


# revision 30
# speedup vs baseline: 1.2162x; 1.2162x over previous
# Discrete-Hawkes kernel, v6: windowed-decay direct query evaluation.
#
# lam(t,s) = relu( mu[s] + beta * H[t,s] ),
#   H[t,s] = sum_{tp<t} a^(t-tp) * c[tp,s],  c = obs @ alpha,  a = exp(-beta)
#
# Key identity: swap the contraction order. Only B = 8192 grid points are
# read, so never form c/H on the [T,S] grid:
#
#   out[i] = relu( mu[s_i] + sum_j sum_sp G[i,j] * obs[j,sp] * alpha[sp,s_i] )
#   G[i,j] = beta * a^(t_i - j)  for 0 < t_i - j, truncated to a >=64-step
#            window (a^64 = exp(-64*beta) <= 1.7e-3 relative, beta >= 0.1).
#
# Per core (1024 queries, sorted by t, 9 query tiles of 128):
#  * Q = G^T-matmuls over a 10-tile obs slab: per query tile 3 j-tiles x
#    2 PSUM halves (bf16 G^T stationary, fp8 obs moving -- mixed dtypes).
#  * out[i] = relu(dot(Q[i,:], alphaT[s_i,:]) + mu[s_i]): DVE elementwise
#    product vs gathered alphaT rows, free-axis accumulate on ScalarE's
#    activation accumulator (one tile on DVE), relu+mu batched on DVE.
#  * All gathers (G rows by t_i shift, alphaT rows by s_i, mu by s_i) and
#    the final inverse permutation are host-side index prep; beta is
#    folded into the G table so no runtime scalars reach the device.
#
# DMA (~3.4MB/core): everything fp8 except G^T (bf16). Transfers are
# split per query tile / per j-tile and issued in consumption order,
# balanced over the three issue queues (sync + scalar HWDGE, gpsimd
# SWDGE), so each tile's gates land just-in-time. A stream of small
# dummy matmuls keeps the PE p-state ramped while the head DMAs land.

import numpy as np
import ml_dtypes

T, S, B = 8192, 1024, 8192
NCORES = 8
P = 128
WIN = 64            # guaranteed decay lookback per query
JW = 384            # G row width = 3 j-tiles
QT = 9              # query tiles per core (1152 slots for 1024 queries)
ANCH = (0, 1, 2, 3, 4, 5, 6, 7, 7)   # j-tile anchor per query tile
NJ = 10             # obs j-tiles per core (1280 rows)
NQ = B // NCORES    # queries per core
N_DUMMY = 56        # PE p-state keep-alive matmuls (64 cols each)
DVE_RED = (8,)      # q-tiles whose accumulate runs as a DVE reduce

_NC_CACHE = {}
LAST_RESULT = None


def _build():
    if "nc" in _NC_CACHE:
        return _NC_CACHE["nc"]

    import concourse.mybir as mybir
    import concourse.tile as tile
    from concourse import bacc

    dt = mybir.dt
    nc = bacc.Bacc("TRN2", target_bir_lowering=False, debug=False,
                   num_devices=NCORES)

    gt_d = [nc.dram_tensor(f"gt{g}", [P, 3, P], dt.bfloat16,
                           kind="ExternalInput") for g in range(QT)]
    obs_d = [nc.dram_tensor(f"obs{j}", [P, S], dt.float8e4,
                            kind="ExternalInput") for j in range(NJ)]
    ag_d = [nc.dram_tensor(f"ag{g}", [P, S], dt.float8e4,
                           kind="ExternalInput") for g in range(QT)]
    fl_d = nc.dram_tensor("fl", [P, QT], dt.float32, kind="ExternalInput")
    lam_d = nc.dram_tensor("lam", [P, QT], dt.float32, kind="ExternalOutput")

    with tile.TileContext(nc) as tc:
        with (
            tc.tile_pool(name="inp", bufs=1) as inp,
            tc.tile_pool(name="psq", bufs=3, space="PSUM") as psq_pool,
            tc.tile_pool(name="psd", bufs=1, space="PSUM") as psd_pool,
            tc.tile_pool(name="scr", bufs=4) as scr_pool,
        ):
            dmy = inp.tile([P, 64], dt.bfloat16, tag="dmy")
            nc.vector.memset(dmy[:], 0)
            psd = psd_pool.tile([P, 64], dt.float32, tag="psd")

            fl_sb = inp.tile([P, QT], dt.float32, tag="fl")
            gt_sb = [inp.tile([P, 3, P], dt.bfloat16, tag=f"gt{g}",
                              name=f"gt{g}") for g in range(QT)]
            obs_sb = [inp.tile([P, S], dt.float8e4, tag=f"obs{j}",
                               name=f"obs{j}") for j in range(NJ)]
            ag_sb = [inp.tile([P, S], dt.float8e4, tag=f"ag{g}",
                              name=f"ag{g}") for g in range(QT)]
            lam_sb = inp.tile([P, QT], dt.float32, tag="lam")
            acc_sb = inp.tile([P, QT], dt.float32, tag="acc")
            scr2 = inp.tile([P, S], dt.bfloat16, tag="scr2")

            # Consumption-ordered waves over three queues:
            #   sync:   gt0..gt8, obs9       (1.01 MB)
            #   scalar: obs0..obs8           (1.18 MB)
            #   gpsimd: fl, ag0..ag8         (1.19 MB)
            nc.sync.dma_start(gt_sb[0][:], gt_d[0][:])
            nc.scalar.dma_start(obs_sb[0][:], obs_d[0][:])
            nc.gpsimd.dma_start(fl_sb[:], fl_d[:])
            nc.scalar.dma_start(obs_sb[1][:], obs_d[1][:])
            nc.scalar.dma_start(obs_sb[2][:], obs_d[2][:])
            for g in range(1, QT):
                nc.sync.dma_start(gt_sb[g][:], gt_d[g][:])
                if g + 2 < NJ - 1:
                    nc.scalar.dma_start(obs_sb[g + 2][:], obs_d[g + 2][:])
                nc.gpsimd.dma_start(ag_sb[g - 1][:], ag_d[g - 1][:])
            nc.sync.dma_start(obs_sb[NJ - 1][:], obs_d[NJ - 1][:])
            nc.gpsimd.dma_start(ag_sb[QT - 1][:], ag_d[QT - 1][:])

            # PE clock keep-alive while the head DMAs land.
            for _ in range(N_DUMMY):
                nc.tensor.matmul(psd[0:64, :], dmy[:, 0:64], dmy[:, :],
                                 start=True, stop=True)

            def qtile(g):
                psq = psq_pool.tile([P, S], dt.float32, tag="psq",
                                    name=f"psq{g}")
                # PSUM matmul groups are per-bank: two 512-wide halves.
                for h in range(2):
                    for c in range(3):
                        ob = obs_sb[ANCH[g] + c]
                        nc.tensor.matmul(
                            psq[:, h * 512:(h + 1) * 512],
                            gt_sb[g][:, c, :],
                            ob[:, h * 512:(h + 1) * 512],
                            start=(c == 0), stop=(c == 2))
                scr = scr_pool.tile([P, S], dt.bfloat16, tag="scr",
                                    name=f"scr{g}")
                # dot(Q[i,:], alphaT[s_i,:]): product on DVE (only engine
                # that reads PSUM fast); accumulate on ScalarE's activation
                # accumulator (last tile on DVE to shorten the tail).
                nc.vector.tensor_tensor(out=scr[:], in0=psq[:],
                                        in1=ag_sb[g][:],
                                        op=mybir.AluOpType.mult)
                if g in DVE_RED:
                    nc.vector.tensor_reduce(out=acc_sb[:, g:g + 1],
                                            in_=scr[:],
                                            axis=mybir.AxisListType.X,
                                            op=mybir.AluOpType.add)
                else:
                    nc.scalar.activation(scr2[:], scr[:],
                                         mybir.ActivationFunctionType.Copy,
                                         accum_out=acc_sb[:, g:g + 1])

            for g in range(QT):
                qtile(g)

            # lam = relu(acc + mu[s_i]) for all tiles at once (beta is
            # folded into G); two small DVE ops instead of 9 activations.
            nc.vector.tensor_tensor(out=lam_sb[:], in0=acc_sb[:],
                                    in1=fl_sb[:], op=mybir.AluOpType.add)
            nc.vector.tensor_scalar_max(lam_sb[:], lam_sb[:], 0.0)
            nc.sync.dma_start(lam_d[:], lam_sb[:])

    nc.compile()
    _NC_CACHE["nc"] = nc
    return nc


def _assign(tc):
    """Greedy assignment of one core's sorted t values to the QT tiles.

    Tile g may hold query t iff its 384-wide obs window [A0+128*ANCH[g],
    ... +JW) covers [max(0, t-WIN), t]. Returns (A0, per-tile index lists).
    """
    tmin, tmax = int(tc[0]), int(tc[-1])
    span = tmax - tmin
    top = 128 * ANCH[-1] + JW - 1
    cands = [tmin - WIN - max(0, (top - WIN - span)) // 2]
    cands += list(range(tmax - top, tmin - WIN + 1, 32))
    for A0 in cands:
        if tmax > A0 + top:
            continue
        assign = [[] for _ in range(QT)]
        g = 0
        ok = True
        for qi in range(len(tc)):
            tv = int(tc[qi])
            while g < QT and not (
                    max(0, tv - WIN) >= A0 + 128 * ANCH[g]
                    and tv <= A0 + 128 * ANCH[g] + JW - 1
                    and len(assign[g]) < P):
                g += 1
            if g == QT:
                ok = False
                break
            assign[g].append(qi)
        if ok:
            return A0, assign
    raise RuntimeError("no feasible window placement for this t distribution")


def _prep_inputs(t, s, obs, alpha, beta, mu):
    bf16 = ml_dtypes.bfloat16
    fp8 = ml_dtypes.float8_e4m3fn
    t_i = np.asarray(t).astype(np.int64)
    s_i = np.asarray(s).astype(np.int64)
    beta32 = np.float32(np.asarray(beta).reshape(-1)[0])
    a64 = np.exp(-np.float64(beta32))

    # Decay table, one row per shift u: gtab[u, j] = beta * a^(384-u-j) for
    # 1 <= 384-u-j < 384, else 0. A query t in tile g (core offset A0) uses
    # row u = 384 - (t - A0 - 128*ANCH[g]); row 384 is all zero (pad slots).
    uu = np.arange(385, dtype=np.int64)[:, None]
    jj = np.arange(JW, dtype=np.int64)[None, :]
    x = 384 - uu - jj
    tab = np.where((x >= 1) & (x < 384), a64 ** np.clip(x, 0, 400), 0.0)
    gtab = (beta32 * tab).astype(np.float32).astype(bf16)

    obs_f8 = np.asarray(obs).astype(fp8)
    alphat = np.ascontiguousarray(
        np.asarray(alpha, dtype=np.float32).T).astype(fp8)
    mu32 = np.asarray(mu, dtype=np.float32)

    order = np.argsort(t_i, kind="stable")
    in_maps = []
    slot_q = np.full((NCORES, P, QT), -1, dtype=np.int64)
    for k in range(NCORES):
        idx = order[k * NQ:(k + 1) * NQ]
        tc, sc = t_i[idx], s_i[idx]
        A0, assign = _assign(tc)

        block = np.zeros((NJ * P, S), dtype=fp8)
        lo, hi = max(0, A0), min(T, A0 + NJ * P)
        block[lo - A0:hi - A0] = obs_f8[lo:hi]
        slab = block.reshape(NJ, P, S)

        uoff = np.full((P, QT), 384, dtype=np.int64)  # 384 = all-zero row
        soff = np.zeros((P, QT), dtype=np.int64)
        fl = np.zeros((P, QT), dtype=np.float32)
        for g, lst in enumerate(assign):
            for p, qi in enumerate(lst):
                tv, sv = int(tc[qi]), int(sc[qi])
                uoff[p, g] = 384 - (tv - A0 - 128 * ANCH[g])
                soff[p, g] = sv
                fl[p, g] = mu32[sv]
                slot_q[k, p, g] = idx[qi]
        g_rows = gtab[uoff]                       # [P(q), QT, JW]
        ag_rows = alphat[soff]                    # [P, QT, S]
        ag_rows[uoff == 384] = 0                  # pad slots contribute 0
        # pre-transposed G chunks: gt[g][j, c, q] = g_rows[q, g, c*128+j]
        gt = g_rows.reshape(P, QT, 3, P).transpose(1, 3, 2, 0)

        im = {"fl": fl}
        for g in range(QT):
            im[f"gt{g}"] = np.ascontiguousarray(gt[g])
            im[f"ag{g}"] = np.ascontiguousarray(ag_rows[:, g])
        for j in range(NJ):
            im[f"obs{j}"] = np.ascontiguousarray(slab[j])
        in_maps.append(im)
    return in_maps, slot_q


def kernel(t, s, obs, alpha, beta, mu):
    global LAST_RESULT
    from concourse import bass_utils

    nc = _build()
    in_maps, slot_q = _prep_inputs(t, s, obs, alpha, beta, mu)
    res = bass_utils.run_bass_kernel_spmd(nc, in_maps,
                                          core_ids=list(range(NCORES)))
    LAST_RESULT = res

    lam = np.stack([np.asarray(r["lam"], dtype=np.float32)
                    for r in res.results])        # [NCORES, P, QT]
    out = np.zeros(B, dtype=np.float32)
    valid = slot_q >= 0
    out[slot_q[valid]] = lam[valid]
    return np.ascontiguousarray(out)


# revision 31
# speedup vs baseline: 1.2246x; 1.0069x over previous
# Discrete-Hawkes kernel, v6: windowed-decay direct query evaluation.
#
# lam(t,s) = relu( mu[s] + beta * H[t,s] ),
#   H[t,s] = sum_{tp<t} a^(t-tp) * c[tp,s],  c = obs @ alpha,  a = exp(-beta)
#
# Key identity: swap the contraction order. Only B = 8192 grid points are
# read, so never form c/H on the [T,S] grid:
#
#   out[i] = relu( mu[s_i] + sum_j sum_sp G[i,j] * obs[j,sp] * alpha[sp,s_i] )
#   G[i,j] = beta * a^(t_i - j)  for 0 < t_i - j, truncated to a >=64-step
#            window (a^64 = exp(-64*beta) <= 1.7e-3 relative, beta >= 0.1).
#
# Per core (1024 queries, sorted by t, 9 query tiles of 128):
#  * Q = G^T-matmuls over a 10-tile obs slab: per query tile 3 j-tiles x
#    2 PSUM halves (bf16 G^T stationary, fp8 obs moving -- mixed dtypes).
#  * out[i] = relu(dot(Q[i,:], alphaT[s_i,:]) + mu[s_i]): DVE elementwise
#    product vs gathered alphaT rows, free-axis accumulate on ScalarE's
#    activation accumulator (one tile on DVE), relu+mu batched on DVE.
#  * All gathers (G rows by t_i shift, alphaT rows by s_i, mu by s_i) and
#    the final inverse permutation are host-side index prep; beta is
#    folded into the G table so no runtime scalars reach the device.
#
# DMA (~3.4MB/core): everything fp8 except G^T (bf16). Transfers are
# split per query tile / per j-tile and issued in consumption order,
# balanced over the three issue queues (sync + scalar HWDGE, gpsimd
# SWDGE), so each tile's gates land just-in-time. A stream of small
# dummy matmuls keeps the PE p-state ramped while the head DMAs land.

import numpy as np
import ml_dtypes

T, S, B = 8192, 1024, 8192
NCORES = 8
P = 128
WIN = 64            # guaranteed decay lookback per query
JW = 384            # G row width = 3 j-tiles
QT = 9              # query tiles per core (1152 slots for 1024 queries)
ANCH = (0, 1, 2, 3, 4, 5, 6, 7, 7)   # j-tile anchor per query tile
NJ = 10             # obs j-tiles per core (1280 rows)
NQ = B // NCORES    # queries per core
N_DUMMY = 56        # PE p-state keep-alive matmuls (64 cols each)
DVE_RED = (8,)      # q-tiles whose accumulate runs as a DVE reduce

_NC_CACHE = {}
LAST_RESULT = None


def _build():
    if "nc" in _NC_CACHE:
        return _NC_CACHE["nc"]

    import concourse.mybir as mybir
    import concourse.tile as tile
    from concourse import bacc

    dt = mybir.dt
    nc = bacc.Bacc("TRN2", target_bir_lowering=False, debug=False,
                   num_devices=NCORES)

    gt_d = [nc.dram_tensor(f"gt{g}", [P, 3, P], dt.bfloat16,
                           kind="ExternalInput") for g in range(QT)]
    obs_d = [nc.dram_tensor(f"obs{j}", [P, S], dt.float8e4,
                            kind="ExternalInput") for j in range(NJ)]
    ag_d = [nc.dram_tensor(f"ag{g}", [P, S], dt.float8e4,
                           kind="ExternalInput") for g in range(QT)]
    fl_d = nc.dram_tensor("fl", [P, QT], dt.float32, kind="ExternalInput")
    lam_d = nc.dram_tensor("lam", [P, QT], dt.float32, kind="ExternalOutput")

    with tile.TileContext(nc) as tc:
        with (
            tc.tile_pool(name="inp", bufs=1) as inp,
            tc.tile_pool(name="psq", bufs=3, space="PSUM") as psq_pool,
            tc.tile_pool(name="psd", bufs=1, space="PSUM") as psd_pool,
            tc.tile_pool(name="scr", bufs=4) as scr_pool,
        ):
            dmy = inp.tile([P, 64], dt.bfloat16, tag="dmy")
            nc.vector.memset(dmy[:], 0)
            psd = psd_pool.tile([P, 64], dt.float32, tag="psd")

            fl_sb = inp.tile([P, QT], dt.float32, tag="fl")
            gt_sb = [inp.tile([P, 3, P], dt.bfloat16, tag=f"gt{g}",
                              name=f"gt{g}") for g in range(QT)]
            obs_sb = [inp.tile([P, S], dt.float8e4, tag=f"obs{j}",
                               name=f"obs{j}") for j in range(NJ)]
            ag_sb = [inp.tile([P, S], dt.float8e4, tag=f"ag{g}",
                              name=f"ag{g}") for g in range(QT)]
            lam_sb = inp.tile([P, QT], dt.float32, tag="lam")
            acc_sb = inp.tile([P, QT], dt.float32, tag="acc")
            scr2 = inp.tile([P, S], dt.bfloat16, tag="scr2")

            # Consumption-ordered waves over three queues:
            #   sync:   gt0..gt8, obs9       (1.01 MB)
            #   scalar: obs0..obs8           (1.18 MB)
            #   gpsimd: fl, ag0..ag8         (1.19 MB)
            nc.sync.dma_start(gt_sb[0][:], gt_d[0][:])
            nc.scalar.dma_start(obs_sb[0][:], obs_d[0][:])
            nc.gpsimd.dma_start(fl_sb[:], fl_d[:])
            nc.scalar.dma_start(obs_sb[1][:], obs_d[1][:])
            nc.scalar.dma_start(obs_sb[2][:], obs_d[2][:])
            for g in range(1, QT):
                nc.sync.dma_start(gt_sb[g][:], gt_d[g][:])
                if g + 2 < NJ - 1:
                    nc.scalar.dma_start(obs_sb[g + 2][:], obs_d[g + 2][:])
                nc.gpsimd.dma_start(ag_sb[g - 1][:], ag_d[g - 1][:])
            nc.sync.dma_start(obs_sb[NJ - 1][:], obs_d[NJ - 1][:])
            nc.gpsimd.dma_start(ag_sb[QT - 1][:], ag_d[QT - 1][:])

            # PE clock keep-alive while the head DMAs land.
            for _ in range(N_DUMMY):
                nc.tensor.matmul(psd[0:64, :], dmy[:, 0:64], dmy[:, :],
                                 start=True, stop=True)

            def qtile(g):
                psq = psq_pool.tile([P, S], dt.float32, tag="psq",
                                    name=f"psq{g}")
                # PSUM matmul groups are per-bank: two 512-wide halves,
                # interleaved c-outer so matmul #1 gates only on obs[anchor].
                for c in range(3):
                    for h in range(2):
                        ob = obs_sb[ANCH[g] + c]
                        nc.tensor.matmul(
                            psq[:, h * 512:(h + 1) * 512],
                            gt_sb[g][:, c, :],
                            ob[:, h * 512:(h + 1) * 512],
                            start=(c == 0), stop=(c == 2))
                scr = scr_pool.tile([P, S], dt.bfloat16, tag="scr",
                                    name=f"scr{g}")
                # dot(Q[i,:], alphaT[s_i,:]): product on DVE (only engine
                # that reads PSUM fast); accumulate on ScalarE's activation
                # accumulator (last tile on DVE to shorten the tail).
                nc.vector.tensor_tensor(out=scr[:], in0=psq[:],
                                        in1=ag_sb[g][:],
                                        op=mybir.AluOpType.mult)
                if g in DVE_RED:
                    nc.vector.tensor_reduce(out=acc_sb[:, g:g + 1],
                                            in_=scr[:],
                                            axis=mybir.AxisListType.X,
                                            op=mybir.AluOpType.add)
                else:
                    nc.scalar.activation(scr2[:], scr[:],
                                         mybir.ActivationFunctionType.Copy,
                                         accum_out=acc_sb[:, g:g + 1])

            for g in range(QT):
                qtile(g)

            # lam = relu(acc + mu[s_i]) for all tiles at once (beta is
            # folded into G); two small DVE ops instead of 9 activations.
            nc.vector.tensor_tensor(out=lam_sb[:], in0=acc_sb[:],
                                    in1=fl_sb[:], op=mybir.AluOpType.add)
            nc.vector.tensor_scalar_max(lam_sb[:], lam_sb[:], 0.0)
            nc.sync.dma_start(lam_d[:], lam_sb[:])

    nc.compile()
    _NC_CACHE["nc"] = nc
    return nc


def _assign(tc):
    """Greedy assignment of one core's sorted t values to the QT tiles.

    Tile g may hold query t iff its 384-wide obs window [A0+128*ANCH[g],
    ... +JW) covers [max(0, t-WIN), t]. Returns (A0, per-tile index lists).
    """
    tmin, tmax = int(tc[0]), int(tc[-1])
    span = tmax - tmin
    top = 128 * ANCH[-1] + JW - 1
    cands = [tmin - WIN - max(0, (top - WIN - span)) // 2]
    cands += list(range(tmax - top, tmin - WIN + 1, 32))
    for A0 in cands:
        if tmax > A0 + top:
            continue
        assign = [[] for _ in range(QT)]
        g = 0
        ok = True
        for qi in range(len(tc)):
            tv = int(tc[qi])
            while g < QT and not (
                    max(0, tv - WIN) >= A0 + 128 * ANCH[g]
                    and tv <= A0 + 128 * ANCH[g] + JW - 1
                    and len(assign[g]) < P):
                g += 1
            if g == QT:
                ok = False
                break
            assign[g].append(qi)
        if ok:
            return A0, assign
    raise RuntimeError("no feasible window placement for this t distribution")


def _prep_inputs(t, s, obs, alpha, beta, mu):
    bf16 = ml_dtypes.bfloat16
    fp8 = ml_dtypes.float8_e4m3fn
    t_i = np.asarray(t).astype(np.int64)
    s_i = np.asarray(s).astype(np.int64)
    beta32 = np.float32(np.asarray(beta).reshape(-1)[0])
    a64 = np.exp(-np.float64(beta32))

    # Decay table, one row per shift u: gtab[u, j] = beta * a^(384-u-j) for
    # 1 <= 384-u-j < 384, else 0. A query t in tile g (core offset A0) uses
    # row u = 384 - (t - A0 - 128*ANCH[g]); row 384 is all zero (pad slots).
    uu = np.arange(385, dtype=np.int64)[:, None]
    jj = np.arange(JW, dtype=np.int64)[None, :]
    x = 384 - uu - jj
    tab = np.where((x >= 1) & (x < 384), a64 ** np.clip(x, 0, 400), 0.0)
    gtab = (beta32 * tab).astype(np.float32).astype(bf16)

    obs_f8 = np.asarray(obs).astype(fp8)
    alphat = np.ascontiguousarray(
        np.asarray(alpha, dtype=np.float32).T).astype(fp8)
    mu32 = np.asarray(mu, dtype=np.float32)

    order = np.argsort(t_i, kind="stable")
    in_maps = []
    slot_q = np.full((NCORES, P, QT), -1, dtype=np.int64)
    for k in range(NCORES):
        idx = order[k * NQ:(k + 1) * NQ]
        tc, sc = t_i[idx], s_i[idx]
        A0, assign = _assign(tc)

        block = np.zeros((NJ * P, S), dtype=fp8)
        lo, hi = max(0, A0), min(T, A0 + NJ * P)
        block[lo - A0:hi - A0] = obs_f8[lo:hi]
        slab = block.reshape(NJ, P, S)

        uoff = np.full((P, QT), 384, dtype=np.int64)  # 384 = all-zero row
        soff = np.zeros((P, QT), dtype=np.int64)
        fl = np.zeros((P, QT), dtype=np.float32)
        for g, lst in enumerate(assign):
            for p, qi in enumerate(lst):
                tv, sv = int(tc[qi]), int(sc[qi])
                uoff[p, g] = 384 - (tv - A0 - 128 * ANCH[g])
                soff[p, g] = sv
                fl[p, g] = mu32[sv]
                slot_q[k, p, g] = idx[qi]
        g_rows = gtab[uoff]                       # [P(q), QT, JW]
        ag_rows = alphat[soff]                    # [P, QT, S]
        ag_rows[uoff == 384] = 0                  # pad slots contribute 0
        # pre-transposed G chunks: gt[g][j, c, q] = g_rows[q, g, c*128+j]
        gt = g_rows.reshape(P, QT, 3, P).transpose(1, 3, 2, 0)

        im = {"fl": fl}
        for g in range(QT):
            im[f"gt{g}"] = np.ascontiguousarray(gt[g])
            im[f"ag{g}"] = np.ascontiguousarray(ag_rows[:, g])
        for j in range(NJ):
            im[f"obs{j}"] = np.ascontiguousarray(slab[j])
        in_maps.append(im)
    return in_maps, slot_q


def kernel(t, s, obs, alpha, beta, mu):
    global LAST_RESULT
    from concourse import bass_utils

    nc = _build()
    in_maps, slot_q = _prep_inputs(t, s, obs, alpha, beta, mu)
    res = bass_utils.run_bass_kernel_spmd(nc, in_maps,
                                          core_ids=list(range(NCORES)))
    LAST_RESULT = res

    lam = np.stack([np.asarray(r["lam"], dtype=np.float32)
                    for r in res.results])        # [NCORES, P, QT]
    out = np.zeros(B, dtype=np.float32)
    valid = slot_q >= 0
    out[slot_q[valid]] = lam[valid]
    return np.ascontiguousarray(out)
